# revision 22
# baseline (speedup 1.0000x reference)
"""Trainium2 Bass kernel for nn_MemoryModel (delta-rule memory scan).

Mathematical reduction (derivation):
  The encoder is position-local, so hidden[b,t] = f(seq[b,t]) takes only
  VOCAB=64 distinct values -> a (64, 32) table computed on host from the
  (tiny) parameter tensors.

  The reference forward scan M_{t+1} = M_t A_t + B_t with
    A_t = I - k k^T / (k.k + 1e-6),  B_t = k k^T
  only feeds the output through ctx = M_final @ q.  Running the affine
  recurrence ADJOINT (backward over steps, u_0 = q):
    c_j   = k_j . u_j
    ctx  += k_j c_j
    u_j+1 = u_j - (k_j / d_j) c_j
  gives ctx exactly, turning the (B,32,32) matrix scan into a (B,32)
  vector scan: O(B*L*H) instead of O(B*L*H^2).

  Finally out = ctx @ (wo wr)^T + (br wo^T + bo).

Device mapping (per core, pure data parallel over batch):
  - 256 batches/core as 2 column-packed 128-partition tiles.
  - Per (batch, step) [k | k/d] bf16 rows are assembled host-side (a
    64-row table lookup — layout prep) and streamed in chunked DMAs.
  - Scan step:
      DVE  tensor_tensor        tmp = k (*) u          (both tiles packed)
      ACT  activation accum_out negc_t = -sum(tmp_t)   (per tile)
      DVE  scalar_tensor_tensor u_t   += khat_t * negc_t
      DVE  scalar_tensor_tensor ctxn_t += k_t * negc_t
  - Final projection on PE: transpose ctxneg, append ones row, single
    33x128 @ 33x64 matmul folding -(wo wr)^T and the bias, DMA out.
"""

import os
import sys
from contextlib import ExitStack

import numpy as np

for _p in ("/opt/trn_rl_repo", "/root/.axon_site/_ro/trn_rl_repo"):
    if os.path.isdir(_p) and _p not in sys.path:
        sys.path.insert(0, _p)

import ml_dtypes  # noqa: E402
import concourse.bass as bass  # noqa: E402
import concourse.tile as tile  # noqa: E402
import concourse.mybir as mybir  # noqa: E402
from concourse import bass_utils  # noqa: E402

# ---- problem constants (hardcoded per contest contract) ----
B, L, H, V = 2048, 1024, 32, 64
NCORES = 8
NB = B // NCORES          # 256 batches per core
NTILE = NB // 128         # 2 tiles of 128 partitions
F32 = mybir.dt.float32
BF16 = mybir.dt.bfloat16
MULT = mybir.AluOpType.mult
ADD = mybir.AluOpType.add
AXX = mybir.AxisListType.X
ACT_COPY = mybir.ActivationFunctionType.Copy

CH = 64                   # slots per streamed chunk


def _host_tables(embed, w1, b1, w2, b2, ln_g, ln_b, wr, br, wo, bo):
    """Tiny parameter-only precompute (float64 on host)."""
    h = embed.astype(np.float64)
    ff = np.maximum(h @ w1.T.astype(np.float64) + b1, 0) @ w2.T.astype(np.float64) + b2
    x = h + ff
    mu = x.mean(-1, keepdims=True)
    var = x.var(-1, keepdims=True)
    table = (x - mu) / np.sqrt(var + 1e-5) * ln_g + ln_b          # (64, 32)
    d = (table ** 2).sum(-1) + 1e-6
    that = table / d[:, None]
    kk = np.concatenate([table, that], axis=1)                     # (64,64) [k|khat]
    kk_bf = kk.astype(ml_dtypes.bfloat16)
    # output projection with ctx = -ctxneg folded in, bias via ones-row trick
    MH = (wo.astype(np.float64) @ wr.astype(np.float64)).T         # (32, 64)
    const = br.astype(np.float64) @ wo.T.astype(np.float64) + bo
    maug = np.zeros((H + 1, V), np.float32)
    maug[:H] = -MH
    maug[H] = const
    return kk_bf, maug


def _host_kd(seq_core, kk_bf, nslot=L, ch=CH):
    """Streamed [k|khat] rows for one core, chunk-major layout.

    out[c, p, (s*NTILE+t)*64 + e] = kk_bf[seq_rev[t*128+p, c*ch+s], e]
    Slot v uses token seq[:, L-1-v]; slot 0 is the query (u init).
    """
    nchunk = nslot // ch
    srev = seq_core[:, ::-1][:, :nslot]
    rows = kk_bf[srev]                                # (NB, nslot, 64) bf16
    r = rows.reshape(NTILE, 128, nchunk, ch, 2 * H)
    out = r.transpose(2, 1, 3, 0, 4)                  # (nchunk,128,ch,NTILE,64)
    return np.ascontiguousarray(out.reshape(nchunk, 128, ch * NTILE * 2 * H))


def _split_long_waits(nc, maxw=1):
    """Walrus (bass2jax/axon path) rejects instructions carrying more than
    one semaphore wait ("Too many sync wait commands") — notably the Tile
    exit drain, which waits on every live semaphore. Peel excess waits onto
    same-engine NoOps inserted immediately before the offender."""
    for fn in nc.m.functions:
        for blk in fn.blocks:
            new_insts = []
            for inst in blk.instructions:
                si = inst.sync_info
                if si is not None and len(si.on_wait) > maxw:
                    waits = list(si.on_wait)
                    n_extra = 0
                    while len(waits) > maxw:
                        head, waits = waits[:maxw], waits[maxw:]
                        nop = mybir.InstNoOp(
                            name=f"{inst.name}_ws{n_extra}",
                            sync_info=mybir.SyncInfo(on_wait=head, on_update=[]),
                            engine=inst.engine,
                            bass_nofuse=True,
                        )
                        n_extra += 1
                        nc.register_instruction(nop, overwrite=True)
                        new_insts.append(nop)
                    si.on_wait = waits
                new_insts.append(inst)
            blk.instructions[:] = new_insts


def build_nc_crit(nslot=L, ch=CH, dma_split=4, repeat=1, nsbuf=3):
    """Critical-section build: the scan runs as one Tile critical section.

    All scan ops execute on DVE in program order (same-engine RAW is safe:
    the DVE drains each op), so Tile's per-dependent-op semaphores are not
    needed; DMA double-buffering is coordinated with two manual semaphores
    (dma_sem: DMA -> DVE chunk ready; scan_sem: DVE -> DMA slot free).
    """
    nchunk = nslot // ch
    fdim = ch * NTILE * 2 * H
    nc = bass.Bass(
        "TRN2",
        target_bir_lowering=False,
        debug=False,
        enable_asserts=False,
        num_devices=NCORES,
    )
    kd = nc.dram_tensor("kd", [nchunk, 128, fdim], BF16, kind="ExternalInput")
    maug = nc.dram_tensor("maug", [H + 1, V], F32, kind="ExternalInput")
    ident = nc.dram_tensor("ident", [128, 128], F32, kind="ExternalInput")
    out = nc.dram_tensor("out", [NB, V], F32, kind="ExternalOutput")

    with tile.TileContext(nc) as tc, ExitStack() as ctx:
        const_pool = ctx.enter_context(tc.tile_pool(name="const", bufs=1))
        state_pool = ctx.enter_context(tc.tile_pool(name="state", bufs=1))
        psum_pool = ctx.enter_context(tc.tile_pool(name="ps", bufs=2, space="PSUM"))
        outp = ctx.enter_context(tc.tile_pool(name="outp", bufs=2))

        maug_sb = const_pool.tile([H + 1, V], F32)
        nc.sync.dma_start(maug_sb[:], maug.ap())
        ident_sb = const_pool.tile([128, 128], F32)
        nc.sync.dma_start(ident_sb[:], ident.ap())

        sts = [
            state_pool.tile([128, 2 * H], F32, name=f"st{t}", tag=f"st{t}")
            for t in range(NTILE)
        ]
        for t in range(NTILE):
            nc.vector.memset(sts[t][:, 0:H], 0.0)
        scr = state_pool.tile([128, H], F32, name="scr")
        NRING = 4
        negs = [
            state_pool.tile([128, 1], F32, name=f"negr{i}", tag=f"negr{i}")
            for i in range(NRING)
        ]
        raws = [
            state_pool.tile(
                [128, ch, NTILE, 2 * H], BF16, name=f"rawb{i}", tag=f"rawb{i}"
            )
            for i in range(nsbuf)
        ]

        dma_sem = nc.alloc_semaphore("dma_sem")
        scan_sem = nc.alloc_semaphore("scan_sem")
        sl = ch // dma_split
        fsl = sl * NTILE * 2 * H

        with tc.tile_critical(no_gpsimd_drain=True):
            def issue_dma(d):
                dd = d % nchunk
                for di in range(dma_split):
                    ins = nc.sync.dma_start(
                        raws[d % nsbuf][:, di * sl : (di + 1) * sl, :, :],
                        kd.ap()[dd, :, di * fsl : (di + 1) * fsl],
                    ).then_inc(dma_sem, 16)
                    if di == 0 and d >= nsbuf:
                        # slot reuse: wait until DVE finished chunk d-nsbuf
                        ins._wait_ge(scan_sem, d - nsbuf + 1)

            ntot = nchunk * repeat
            for d in range(min(nsbuf, ntot)):
                issue_dma(d)
            step = 0
            for rep in range(repeat):
              for c in range(nchunk):
                g = rep * nchunk + c
                buf = raws[g % nsbuf]
                first = True

                def mark(ins):
                    nonlocal first
                    if first:
                        ins._wait_ge(dma_sem, 16 * dma_split * (g + 1))
                        first = False
                    return ins

                for s in range(ch):
                    v = c * ch + s
                    if v == 0:
                        for t in range(NTILE):
                            mark(nc.vector.tensor_copy(
                                sts[t][:, H : 2 * H], buf[:, 0, t, 0:H]))
                        continue
                    for t in range(NTILE):
                        ri = (step * NTILE + t) % NRING
                        mark(nc.vector.scalar_tensor_tensor(
                            out=scr[:], in0=buf[:, s, t, 0:H], scalar=-1.0,
                            in1=sts[t][:, H : 2 * H], op0=MULT, op1=MULT,
                            accum_out=negs[ri][:],
                        ))
                    for t in range(NTILE):
                        ri = (step * NTILE + t) % NRING
                        ins = nc.vector.scalar_tensor_tensor(
                            out=sts[t][:], in0=buf[:, s, t, :],
                            scalar=negs[ri][:], in1=sts[t][:],
                            op0=MULT, op1=ADD,
                        )
                        if s == ch - 1 and t == NTILE - 1:
                            ins.then_inc(scan_sem, 1)
                    step += 1
                if g + nsbuf < ntot:
                    issue_dma(g + nsbuf)

        for t in range(NTILE):
            tp = psum_pool.tile([H, 128], F32, tag="tp")
            nc.tensor.transpose(tp[:], sts[t][:, 0:H], ident_sb[:])
            aug = outp.tile([H + 1, 128], F32, tag="aug")
            nc.vector.tensor_copy(aug[0:H, :], tp[:])
            nc.vector.memset(aug[H : H + 1, :], 1.0)
            po = psum_pool.tile([128, V], F32, tag="po")
            nc.tensor.matmul(po[:], aug[:], maug_sb[:])
            ot = outp.tile([128, V], F32, tag="ot")
            nc.vector.tensor_copy(ot[:], po[:])
            nc.sync.dma_start(out.ap()[t * 128 : (t + 1) * 128, :], ot[:])

    _split_long_waits(nc)
    return nc


def build_nc(nslot=L, ch=CH, use_act=True, dma_split=4, probe="", repeat=1,
             stage_copy=False):
    """Build the per-core Bass program (identical across cores)."""
    nchunk = nslot // ch
    fdim = ch * NTILE * 2 * H
    nc = bass.Bass(
        "TRN2",
        target_bir_lowering=False,
        debug=False,
        enable_asserts=False,
        num_devices=NCORES,
    )
    kd = nc.dram_tensor("kd", [nchunk, 128, fdim], BF16, kind="ExternalInput")
    maug = nc.dram_tensor("maug", [H + 1, V], F32, kind="ExternalInput")
    ident = nc.dram_tensor("ident", [128, 128], F32, kind="ExternalInput")
    out = nc.dram_tensor("out", [NB, V], F32, kind="ExternalOutput")

    with tile.TileContext(nc) as tc, ExitStack() as ctx:
        const_pool = ctx.enter_context(tc.tile_pool(name="const", bufs=1))
        state_pool = ctx.enter_context(tc.tile_pool(name="state", bufs=1))
        stream_pool = ctx.enter_context(tc.tile_pool(name="stream", bufs=3))
        work_pool = ctx.enter_context(tc.tile_pool(name="work", bufs=2))
        negc_pool = ctx.enter_context(tc.tile_pool(name="negc", bufs=8))
        scr_pool = ctx.enter_context(tc.tile_pool(name="scr", bufs=8))
        psum_pool = ctx.enter_context(tc.tile_pool(name="ps", bufs=2, space="PSUM"))
        outp = ctx.enter_context(tc.tile_pool(name="outp", bufs=2))

        maug_sb = const_pool.tile([H + 1, V], F32)
        nc.sync.dma_start(maug_sb[:], maug.ap())
        ident_sb = const_pool.tile([128, 128], F32)
        nc.sync.dma_start(ident_sb[:], ident.ap())

        # per-tile state [ctxneg(0:H) | u(H:2H)] — one fused axpy updates both
        sts = [
            state_pool.tile([128, 2 * H], F32, name=f"st{t}", tag=f"st{t}")
            for t in range(NTILE)
        ]
        for t in range(NTILE):
            nc.vector.memset(sts[t][:, 0:H], 0.0)

        # persistent manual rings: avoids per-step pool-release semaphores
        # (all accessors are DVE -> program order, no sync bookkeeping)
        NRING = 8
        scrs = [
            state_pool.tile([128, H], F32, name=f"scrr{i}", tag=f"scrr{i}")
            for i in range(NRING)
        ]
        negs = [
            state_pool.tile([128, 1], F32, name=f"negr{i}", tag=f"negr{i}")
            for i in range(NRING)
        ]
        NSBUF = 3
        raws = [
            state_pool.tile(
                [128, ch, NTILE, 2 * H], BF16, name=f"rawb{i}", tag=f"rawb{i}"
            )
            for i in range(NSBUF)
        ]

        for rep in range(repeat):
          for c in range(nchunk):
            buf = raws[c % NSBUF]
            # split the chunk DMA so multiple queues move it in parallel
            sl = ch // dma_split
            for di in range(dma_split):
                nc.sync.dma_start(
                    buf[:, di * sl : (di + 1) * sl, :, :],
                    kd.ap()[c, :, di * sl * NTILE * 2 * H : (di + 1) * sl * NTILE * 2 * H],
                )
            for s in range(ch):
                v = c * ch + s
                if v == 0:
                    for t in range(NTILE):
                        nc.vector.tensor_copy(sts[t][:, H : 2 * H], buf[:, 0, t, 0:H])
                    continue
                if probe == "dmaonly":
                    continue
                negcs = []
                for t in range(NTILE):
                    # negc = -(k . u); out is scratch
                    ri = (v * NTILE + t) % NRING
                    nc.vector.scalar_tensor_tensor(
                        out=scrs[ri][:], in0=buf[:, s, t, 0:H], scalar=-1.0,
                        in1=sts[t][:, H : 2 * H], op0=MULT, op1=MULT,
                        accum_out=negs[ri][:],
                    )
                    negcs.append(negs[ri])
                if probe == "dotonly":
                    continue
                for t in range(NTILE):
                    # [ctxneg | u] += [k | khat] * negc
                    nc.vector.scalar_tensor_tensor(
                        out=sts[t][:], in0=buf[:, s, t, :], scalar=negcs[t][:],
                        in1=sts[t][:], op0=MULT, op1=ADD,
                    )

        for t in range(NTILE):
            tp = psum_pool.tile([H, 128], F32, tag="tp")
            nc.tensor.transpose(tp[:], sts[t][:, 0:H], ident_sb[:])
            aug = outp.tile([H + 1, 128], F32, tag="aug")
            nc.vector.tensor_copy(aug[0:H, :], tp[:])
            nc.vector.memset(aug[H : H + 1, :], 1.0)
            po = psum_pool.tile([128, V], F32, tag="po")
            nc.tensor.matmul(po[:], aug[:], maug_sb[:])
            ot = outp.tile([128, V], F32, tag="ot")
            nc.vector.tensor_copy(ot[:], po[:])
            nc.sync.dma_start(out.ap()[t * 128 : (t + 1) * 128, :], ot[:])

    _split_long_waits(nc)
    return nc


_CACHED_NC = None


def kernel(seq, embed, w1, b1, w2, b2, ln_g, ln_b, wr, br, wo, bo):
    global _CACHED_NC
    seq = np.asarray(seq)
    kk_bf, maug = _host_tables(
        np.asarray(embed), np.asarray(w1), np.asarray(b1), np.asarray(w2),
        np.asarray(b2), np.asarray(ln_g), np.asarray(ln_b), np.asarray(wr),
        np.asarray(br), np.asarray(wo), np.asarray(bo),
    )
    ident = np.eye(128, dtype=np.float32)
    if _CACHED_NC is None:
        _CACHED_NC = build_nc_crit()
    nc = _CACHED_NC

    in_maps = []
    for core in range(NCORES):
        seq_core = seq[core * NB : (core + 1) * NB]
        in_maps.append(
            {
                "kd": _host_kd(seq_core, kk_bf),
                "maug": maug,
                "ident": ident,
            }
        )
    res = bass_utils.run_bass_kernel_spmd(nc, in_maps, core_ids=list(range(NCORES)))
    out = np.concatenate([res.results[i]["out"] for i in range(NCORES)], axis=0)
    return out.astype(np.float32)
